# revision 1
# baseline (speedup 1.0000x reference)
"""Trainium2 Bass kernel for SAM-style decomposed rel-pos attention.

Problem: B=1, HW=2304 (48x48), NH=16 heads, DH=64, D=1024, f32 in/out.
  attn = softmax(q*scale @ k^T + rel_h[q,kh] + rel_w[q,kw]); out = attn @ v

Strategy (8 NeuronCores, SPMD, no collectives): 2 heads per core.
Host prep: per-core transposed bf16 Q^T (pre-scaled, plus a w-major copy),
K^T; V with a ones-column (softmax denominator falls out of the PV matmul);
reversed rel tables (x8 to cancel the q scale); one-hot Eh/Ew fold the
decomposed bias into the score matmul as extra contraction rows.

Device, phase 1 (both heads, so head 1's DMA gathers overlap head 0's
main loop): T1 = revtable^T @ Q^T (2 x 5 matmuls per head), then 96 tiny
SBUF->SBUF gather DMAs per head (spread over the sync/scalar/gpsimd DMA
rings) materialize rel_h^T / rel_w^T -- DMAs do the per-block diagonal
partition shift engines cannot; one strided copy un-permutes rel_w^T to
q-major so the mm2 moving operand streams contiguously.

Device, phase 2 per head: S^T tiles (128k x qn): mm1 contract 128 =
[Eh | 0 | K^T] x [rel_h^T | 0 | Q^T*s], mm2 adds Ew^T @ rel_w^T; exp on
ScalarE (no max subtraction: scores are O(1)), one exp instruction covers
two k-tiles; PV matmul out^T = V_aug^T @ P^T accumulates over k and issues
one k-pair behind the scores. Explicit scheduler edges keep the PVs AFTER
the next pair's score matmuls in PE order -- otherwise the exp's cumulative
PE-completion wait chains through the PVs and serializes the pipeline.
Normalize by the ones-row via reciprocal + ones-broadcast matmul, DMA out^T
rows to DRAM; host transposes back.
"""

import sys

sys.path.insert(0, "/opt/trn_rl_repo")

import numpy as np
import ml_dtypes

from concourse import bacc, mybir, tile
from concourse.tile import add_dep_helper
from concourse.bass_utils import run_bass_kernel_spmd

BF16 = mybir.dt.bfloat16
F32 = mybir.dt.float32
BF = ml_dtypes.bfloat16

H = 48
W = 48
HW = H * W          # 2304
DH = 64
NH = 16
N_CORES = 8
HPC = 2             # heads per core
KT = HW // 128      # 18 k tiles
QCHUNKS = [(0, 480), (480, 480), (960, 480), (1440, 480), (1920, 384)]

_NC = None


def _build_nc():
    nc = bacc.Bacc(None, target_bir_lowering=False)

    q_t = nc.dram_tensor("q_t", [128, HW], BF16, kind="ExternalInput")
    qw_t = nc.dram_tensor("qw_t", [128, HW], BF16, kind="ExternalInput")
    k_t = nc.dram_tensor("k_t", [128, HW], BF16, kind="ExternalInput")
    v_til = nc.dram_tensor("v_til", [128, HPC * KT * 65], BF16, kind="ExternalInput")
    rhv = nc.dram_tensor("rhv", [64, 95], BF16, kind="ExternalInput")
    rwv = nc.dram_tensor("rwv", [64, 95], BF16, kind="ExternalInput")
    eh = nc.dram_tensor("eh", [64, HW], BF16, kind="ExternalInput")
    ew = nc.dram_tensor("ew", [48, HW], BF16, kind="ExternalInput")
    out_t = nc.dram_tensor("out_t", [128, HW], F32, kind="ExternalOutput")

    Exp = mybir.ActivationFunctionType.Exp

    with tile.TileContext(nc) as tc:
        with (
            tc.tile_pool(name="const", bufs=1) as cpool,
            tc.tile_pool(name="stack", bufs=2) as spool,
            tc.tile_pool(name="ptile", bufs=3) as ppool,
            tc.tile_pool(name="epil", bufs=2) as epool,
            tc.tile_pool(name="ps_s", bufs=2, space="PSUM") as ps_s,
            tc.tile_pool(name="ps_o", bufs=2, space="PSUM") as ps_o,
            tc.tile_pool(name="ps_t1", bufs=1, space="PSUM") as ps_t1,
            tc.tile_pool(name="ps_rb", bufs=1, space="PSUM") as ps_rb,
        ):
            # shared constants; rhv/rwv live at partitions 64:128 to share the
            # base partition of Q^T rows in the stacks (matmul base rule)
            rhv_sb = cpool.tile([128, 95], BF16, tag="rhv")
            rwv_sb = cpool.tile([128, 95], BF16, tag="rwv")
            ew_sb = cpool.tile([48, HW], BF16, tag="ew")
            ones1 = cpool.tile([1, 64], BF16, tag="ones1")
            nc.sync.dma_start(rhv_sb[64:128, :], rhv[:, :])
            nc.sync.dma_start(rwv_sb[64:128, :], rwv[:, :])
            nc.sync.dma_start(ew_sb[:, :], ew[:, :])
            nc.gpsimd.memset(ones1[:], 1.0)

            dma_engines = [nc.sync, nc.scalar, nc.gpsimd]
            heads = []
            # ---- phase 1: prep both heads ----
            for hh in range(HPC):
                c0, c1 = hh * 64, (hh + 1) * 64
                # stacks: rows 0:48 bias block, 48:64 zeros, 64:128 K^T / Q^T
                lhsT = spool.tile([128, HW], BF16, tag="lhsT")
                rhs = spool.tile([128, HW], BF16, tag="rhs")
                qwt = spool.tile([128, HW], BF16, tag="qwt")
                relw = spool.tile([48, HW], BF16, tag="relw")
                relq = spool.tile([48, HW], BF16, tag="relq")
                vt = spool.tile([128, KT * 65], BF16, tag="vt")
                t1h = spool.tile([95, HW], BF16, tag="t1h")
                t2w = spool.tile([95, HW], BF16, tag="t2w")
                nc.sync.dma_start(lhsT[0:64, :], eh[:, :])
                nc.sync.dma_start(lhsT[64:128, :], k_t[c0:c1, :])
                nc.sync.dma_start(rhs[48:64, :], eh[48:64, :])   # zeros
                nc.sync.dma_start(rhs[64:128, :], q_t[c0:c1, :])
                nc.sync.dma_start(qwt[64:128, :], qw_t[c0:c1, :])
                nc.sync.dma_start(vt[:, :], v_til[:, hh * KT * 65 : (hh + 1) * KT * 65])

                # T1h[r, q] = sum_c 8*relh[94-r, c] * qs[c, q]; same for w-major
                for (q0, qn) in QCHUNKS:
                    tp = ps_t1.tile([95, 480], F32, tag="t1")
                    nc.tensor.matmul(
                        tp[:, 0:qn], rhv_sb[64:128, :], rhs[64:128, q0 : q0 + qn],
                        start=True, stop=True,
                    )
                    nc.vector.tensor_copy(t1h[:, q0 : q0 + qn], tp[:, 0:qn])
                for (q0, qn) in QCHUNKS:
                    tp = ps_t1.tile([95, 480], F32, tag="t1")
                    nc.tensor.matmul(
                        tp[:, 0:qn], rwv_sb[64:128, :], qwt[64:128, q0 : q0 + qn],
                        start=True, stop=True,
                    )
                    nc.vector.tensor_copy(t2w[:, q0 : q0 + qn], tp[:, 0:qn])

                # gather diagonals: rel_h^T[j, (h,w)] = T1h[47-h+j, h*48+w]
                for h in range(H):
                    dma_engines[h % 3].dma_start(
                        rhs[0:48, h * 48 : (h + 1) * 48],
                        t1h[47 - h : 95 - h, h * 48 : (h + 1) * 48],
                    )
                # rel_w^T in w-major order: relw[j, w*48+h] = T2w[47-w+j, w*48+h]
                for w in range(W):
                    dma_engines[w % 3].dma_start(
                        relw[0:48, w * 48 : (w + 1) * 48],
                        t2w[47 - w : 95 - w, w * 48 : (w + 1) * 48],
                    )

                # un-permute w-major relw to q-major with one strided copy so
                # the mm2 moving operand streams contiguously
                nc.vector.tensor_copy(
                    relq[:, :].rearrange("p (h w) -> p h w", w=48),
                    relw[:, :].rearrange("p (w h) -> p h w", w=48),
                )
                heads.append((c0, c1, lhsT, rhs, relq, vt))

            # ---- phase 2: main loops ----
            for (c0, c1, lhsT, rhs, relq, vt) in heads:
                for (q0, qn) in QCHUNKS:
                    o_ps = ps_o.tile([65, 480], F32, tag="o")
                    pend = []  # software pipeline: PV issues one k-pair late
                    for kp in range(KT // 2):
                        s_ps = ps_s.tile([128, 1024], F32, tag="s")
                        p_sb = ppool.tile([128, 1024], BF16, tag="p")
                        last_mm = None
                        for half in (0, 1):
                            kt = 2 * kp + half
                            off = half * 512
                            nc.tensor.matmul(
                                s_ps[:, off : off + qn],
                                lhsT[:, kt * 128 : (kt + 1) * 128],
                                rhs[:, q0 : q0 + qn],
                                start=True, stop=False,
                            )
                            last_mm = nc.tensor.matmul(
                                s_ps[:, off : off + qn],
                                ew_sb[:, kt * 128 : (kt + 1) * 128],
                                relq[:, q0 : q0 + qn],
                                start=False, stop=True,
                            )
                        for (pkt, pp, poff) in pend:
                            pv = nc.tensor.matmul(
                                o_ps[:, 0:qn],
                                vt[:, pkt * 65 : (pkt + 1) * 65],
                                pp[:, poff : poff + qn],
                                start=(pkt == 0), stop=(pkt == KT - 1),
                            )
                            # keep PVs after this pair's score mms in PE order:
                            # the exp's cumulative PE wait would otherwise chain
                            # through the PVs and serialize PE<->ACT
                            add_dep_helper(pv.ins, last_mm.ins, sync=False,
                                           reason="pv after score mms")
                        s2 = s_ps[:, :].rearrange("p (b c) -> p b c", b=2)[:, :, 0:qn]
                        p2 = p_sb[:, :].rearrange("p (b c) -> p b c", b=2)[:, :, 0:qn]
                        nc.scalar.activation(p2, s2, Exp)
                        pend = [(2 * kp, p_sb, 0), (2 * kp + 1, p_sb, 512)]
                    for (pkt, pp, poff) in pend:
                        nc.tensor.matmul(
                            o_ps[:, 0:qn],
                            vt[:, pkt * 65 : (pkt + 1) * 65],
                            pp[:, poff : poff + qn],
                            start=(pkt == 0), stop=(pkt == KT - 1),
                        )

                    # normalize: denom = row 64 of o_ps
                    den65 = epool.tile([65, 480], F32, tag="den65")
                    nc.vector.tensor_copy(den65[64:65, 0:qn], o_ps[64:65, 0:qn])
                    den0 = epool.tile([1, 480], F32, tag="den0")
                    nc.sync.dma_start(den0[0:1, 0:qn], den65[64:65, 0:qn])
                    rec0 = epool.tile([1, 480], F32, tag="rec0")
                    nc.vector.reciprocal(rec0[0:1, 0:qn], den0[0:1, 0:qn])
                    recb = epool.tile([1, 480], BF16, tag="recb")
                    nc.vector.tensor_copy(recb[0:1, 0:qn], rec0[0:1, 0:qn])
                    rb_ps = ps_rb.tile([64, 480], F32, tag="rb")
                    nc.tensor.matmul(
                        rb_ps[:, 0:qn], ones1[:], recb[0:1, 0:qn], start=True, stop=True
                    )
                    rb_sb = epool.tile([64, 480], F32, tag="rb_sb")
                    nc.vector.tensor_copy(rb_sb[:, 0:qn], rb_ps[:, 0:qn])
                    ot = epool.tile([64, 480], F32, tag="ot")
                    nc.vector.tensor_mul(ot[:, 0:qn], o_ps[0:64, 0:qn], rb_sb[:, 0:qn])
                    nc.scalar.dma_start(out_t[c0:c1, q0 : q0 + qn], ot[:, 0:qn])

    nc.compile()
    return nc


def _get_nc():
    global _NC
    if _NC is None:
        _NC = _build_nc()
    return _NC


def _host_prep(q, k, v, rel_pos_h, rel_pos_w):
    q2 = np.asarray(q, np.float32).reshape(HW, NH * DH)
    k2 = np.asarray(k, np.float32).reshape(HW, NH * DH)
    v2 = np.asarray(v, np.float32).reshape(HW, NH * DH)
    rph = np.asarray(rel_pos_h, np.float32)
    rpw = np.asarray(rel_pos_w, np.float32)

    ar = np.arange(48)
    # reversed rel tables, x8 cancels the 0.125 q scale
    rhv = np.ascontiguousarray((8.0 * rph[::-1]).T).astype(BF)   # (64, 95)
    rwv = np.ascontiguousarray((8.0 * rpw[::-1]).T).astype(BF)
    kk = np.arange(HW)
    eh = np.zeros((64, HW), np.float32)
    eh[:48] = kk[None, :] // 48 == ar[:, None]
    eh = eh.astype(BF)
    ew = (kk[None, :] % 48 == ar[:, None]).astype(BF)

    onecol = np.ones((HW, 1), np.float32)
    in_maps = []
    for c in range(N_CORES):
        sl = slice(c * 128, (c + 1) * 128)
        qs = (q2[:, sl].T * 0.125).astype(BF)                    # (128, HW)
        qw = np.ascontiguousarray(
            qs.reshape(128, 48, 48).transpose(0, 2, 1)
        ).reshape(128, HW)                                       # w-major cols
        ks = k2[:, sl].T.astype(BF)
        vparts = []
        for hh in range(HPC):
            vh = v2[:, c * 128 + hh * 64 : c * 128 + (hh + 1) * 64]
            va = np.concatenate([vh, onecol], axis=1)            # (HW, 65)
            vparts.append(va.reshape(KT, 128, 65).transpose(1, 0, 2).reshape(128, KT * 65))
        v_til = np.concatenate(vparts, axis=1).astype(BF)        # (128, 2*18*65)
        in_maps.append(
            dict(q_t=qs, qw_t=qw, k_t=ks, v_til=v_til, rhv=rhv, rwv=rwv, eh=eh, ew=ew)
        )
    return in_maps


def _assemble(results):
    cols = [np.asarray(r["out_t"], np.float32).T for r in results]  # (HW, 128) each
    return np.concatenate(cols, axis=1).reshape(1, H, W, NH * DH)


def kernel(q, k, v, rel_pos_h, rel_pos_w):
    nc = _get_nc()
    in_maps = _host_prep(q, k, v, rel_pos_h, rel_pos_w)
    res = run_bass_kernel_spmd(nc, in_maps, core_ids=list(range(N_CORES)))
    return _assemble(res.results)



# revision 7
# speedup vs baseline: 1.7765x; 1.7765x over previous
"""Trainium2 Bass kernel for SAM-style decomposed rel-pos attention (v2).

Problem: B=1, HW=2304 (48x48), NH=16 heads, DH=64, D=1024, f32 in/out.
  attn = softmax(q*scale @ k^T + rel_h[qh,kh] + rel_w[qw,kw]); out = attn @ v

Strategy (8 NeuronCores, SPMD): 2 heads per core. Key ideas vs v1:
- rel_h is folded into the single score matmul per k-tile (one-hot Eh rows
  + K^T stacked as the stationary operand; gathered rel_h^T rows + Q^T as
  the moving operand) -> 18 score matmuls per (head, chunk) instead of 36.
- rel_w is applied MULTIPLICATIVELY after exp: P = exp(S_qk+relh) * Ew
  where Ew[k,q] = exp(rel_w^T[kw(k), q]). Because kw(k) is periodic with
  period 48 and 128 = 2*48 + 32, only 3 row-rotations (offsets 0/32/16 =
  kt mod 3) of exp_relw exist -> a [128, 3, HW] "patterns" tile serves
  every k-tile triple via one DVE tensor_mul per 3-k-tile group.
- The diagonal gathers (rel tables are banded matrices) are done with ONE
  DMA each via a DRAM roundtrip: T1 tables are stored to scratch DRAM and
  re-loaded with a 3D access pattern whose middle dim strides -2256
  (one row up, 48 cols right) -- replacing 96 tiny SBUF DMAs per head.
- exp on ScalarE in [128, 3, qn] groups from PSUM; PV matmuls run 2 groups
  behind the score matmuls so the PE never waits on exp/mul; scheduler
  edges pin PV-after-scores order so the PE instruction stream is gapless
  and the PE_HAM clock gate un-throttles 1.2 -> 2.4 GHz.
- Softmax denominator: ones-column in V_aug -> row 64 of the PV output;
  reciprocal computed on a [96, 5] transposed view (two tiny DMAs) since
  a [1, 480] single-partition reciprocal wastes 127 DVE lanes.
"""

import sys

sys.path.insert(0, "/opt/trn_rl_repo")

import numpy as np
import ml_dtypes

from concourse import bacc, mybir, tile
from concourse.tile import add_dep_helper
from concourse.bass_utils import run_bass_kernel_spmd

BF16 = mybir.dt.bfloat16
F32 = mybir.dt.float32
BF = ml_dtypes.bfloat16

H = 48
W = 48
HW = H * W          # 2304
DH = 64
NH = 16
N_CORES = 8
HPC = 2             # heads per core
KT = HW // 128      # 18 k tiles
QCHUNKS = [(0, 480), (480, 480), (960, 480), (1440, 480), (1920, 384)]
TCHUNKS = [(0, 512), (512, 512), (1024, 512), (1536, 512), (2048, 256)]
NG = KT // 3        # 6 groups of 3 k-tiles
PV_LAG = 3          # PV runs this many groups behind the score matmuls

_NC = None


def _build_nc():
    nc = bacc.Bacc(None, target_bir_lowering=False)

    q_t = nc.dram_tensor("q_t", [128, HW], BF16, kind="ExternalInput")
    qw_t = nc.dram_tensor("qw_t", [128, HW], BF16, kind="ExternalInput")
    k_t = nc.dram_tensor("k_t", [128, HW], BF16, kind="ExternalInput")
    v_til = nc.dram_tensor("v_til", [128, HPC * KT * 65], BF16, kind="ExternalInput")
    rhv = nc.dram_tensor("rhv", [64, 95], BF16, kind="ExternalInput")
    rwv = nc.dram_tensor("rwv", [64, 95], BF16, kind="ExternalInput")
    eh = nc.dram_tensor("eh", [64, HW], BF16, kind="ExternalInput")
    out_t = nc.dram_tensor("out_t", [128, HW], F32, kind="ExternalOutput")
    # scratch DRAM for the diagonal-gather roundtrip (per head)
    t1d = [nc.dram_tensor(f"t1d{h}", [95, HW], BF16, kind="Internal") for h in range(HPC)]
    t2d = [nc.dram_tensor(f"t2d{h}", [95, HW], BF16, kind="Internal") for h in range(HPC)]

    Exp = mybir.ActivationFunctionType.Exp

    with tile.TileContext(nc) as tc:
        with (
            tc.tile_pool(name="const", bufs=1) as cpool,
            tc.tile_pool(name="stack", bufs=2) as spool,
            tc.tile_pool(name="p1t", bufs=3) as p1pool,
            tc.tile_pool(name="p2t", bufs=4) as p2pool,
            tc.tile_pool(name="epil", bufs=2) as epool,
            tc.tile_pool(name="ps_s", bufs=2, space="PSUM") as ps_s,
            tc.tile_pool(name="ps_o", bufs=2, space="PSUM") as ps_o,
        ):
            # shared constants; rhv/rwv live at partitions 64:128 to share the
            # base partition of Q^T rows in the stacks (matmul base rule)
            rhv_sb = cpool.tile([128, 95], BF16, tag="rhv")
            rwv_sb = cpool.tile([128, 95], BF16, tag="rwv")
            ones1 = cpool.tile([1, 64], BF16, tag="ones1")
            nc.sync.dma_start(rhv_sb[64:128, :], rhv[:, :])
            nc.sync.dma_start(rwv_sb[64:128, :], rwv[:, :])
            nc.gpsimd.memset(ones1[:], 1.0)

            heads = []
            # ---- phase 1: prep both heads ----
            for hh in range(HPC):
                dmae = nc.sync if hh == 0 else nc.gpsimd
                c0, c1 = hh * 64, (hh + 1) * 64
                # stacks: rows 0:48 bias block, 48:64 zeros, 64:128 K^T / Q^T
                lhsT = spool.tile([128, HW], BF16, tag="lhsT")
                rhs = spool.tile([128, HW], BF16, tag="rhs")
                qwt = spool.tile([128, HW], BF16, tag="qwt")
                vt = spool.tile([128, KT * 65], BF16, tag="vt")
                t1h = spool.tile([95, HW], BF16, tag="t1h")
                t2w = spool.tile([95, HW], BF16, tag="t2w")
                relw = spool.tile([48, HW], BF16, tag="relw")
                relq = spool.tile([48, HW], BF16, tag="relq")
                expw = spool.tile([48, HW], BF16, tag="expw")
                pats = spool.tile([128, 3, HW], BF16, tag="pats")
                dmae.dma_start(lhsT[0:64, :], eh[:, :])
                dmae.dma_start(lhsT[64:128, :], k_t[c0:c1, :])
                dmae.dma_start(rhs[48:64, :], eh[48:64, :])   # zeros
                dmae.dma_start(rhs[64:128, :], q_t[c0:c1, :])
                dmae.dma_start(qwt[64:128, :], qw_t[c0:c1, :])
                dmae.dma_start(vt[:, :], v_til[:, hh * KT * 65 : (hh + 1) * KT * 65])

                # T1h[r, q] = sum_c 8*relh[94-r, c] * qs[c, q]; T2w same, w-major
                for (q0, qn) in TCHUNKS:
                    tp = ps_s.tile([128, 3, 512], F32, tag="s", name=f"tp_h{hh}_{q0}")
                    nc.tensor.matmul(
                        tp[0:95, 0, 0:qn], rwv_sb[64:128, :], qwt[64:128, q0 : q0 + qn],
                        start=True, stop=True,
                    )
                    nc.vector.tensor_copy(t2w[:, q0 : q0 + qn], tp[0:95, 0, 0:qn])
                for (q0, qn) in TCHUNKS:
                    tp = ps_s.tile([128, 3, 512], F32, tag="s", name=f"tp2_h{hh}_{q0}")
                    nc.tensor.matmul(
                        tp[0:95, 0, 0:qn], rhv_sb[64:128, :], rhs[64:128, q0 : q0 + qn],
                        start=True, stop=True,
                    )
                    nc.vector.tensor_copy(t1h[:, q0 : q0 + qn], tp[0:95, 0, 0:qn])

                # DRAM roundtrip: store tables, re-load with diagonal APs
                dmae.dma_start(t2d[hh][:, :], t2w[:, :])
                dmae.dma_start(t1d[hh][:, :], t1h[:, :])
                # rel_h^T[j, (h,w)] = T1h[47-h+j, h*48+w]  (one DMA)
                dsth = rhs[0:48, :].rearrange("p (h w) -> p h w", w=48)
                srch = t1d[hh][47:95, 0:HW].rearrange("j (h w) -> j h w", w=48)
                srch.ap[1] = [-2256, 48]    # h: one row up, 48 cols right
                dmae.dma_start(dsth, srch)
                # rel_w^T w-major: relw[j, (w,h)] = T2w[47-w+j, w*48+h]
                dstw = relw[0:48, :].rearrange("p (w h) -> p w h", h=48)
                srcw = t2d[hh][47:95, 0:HW].rearrange("j (w h) -> j w h", h=48)
                srcw.ap[1] = [-2256, 48]
                dmae.dma_start(dstw, srcw)

                # un-permute w-major relw to q-major, then exp -> patterns
                nc.vector.tensor_copy(
                    relq[:, :].rearrange("p (h w) -> p h w", w=48),
                    relw[:, :].rearrange("p (w h) -> p h w", w=48),
                )
                nc.scalar.activation(expw[:, :], relq[:, :], Exp)
                # patterns[:, j, :] rows p = expw[(p + off_j) mod 48, :]
                for j, off in enumerate((0, 32, 16)):
                    p = 0
                    ndma = 0
                    while p < 128:
                        s0 = (p + off) % 48
                        n = min(48 - s0, 128 - p)
                        eng = (nc.sync, nc.gpsimd)[(hh + ndma) % 2]
                        eng.dma_start(pats[p : p + n, j, :], expw[s0 : s0 + n, :])
                        p += n
                        ndma += 1
                heads.append((c0, c1, lhsT, rhs, relq, vt, pats))

            # ---- phase 2: main loops ----
            for hi, (c0, c1, lhsT, rhs, relq, vt, pats) in enumerate(heads):
                pend = []   # software pipeline: PV issues PV_LAG groups late
                epiB = []   # deferred epilogue part B (rb matmul waits on recip)
                gctr = [0]  # groups emitted so far

                def epilogue_a(ci, q0, qn, o_ps, hi=hi):
                    # part A: denominator -> reciprocal -> bf16 row (off PE)
                    den = epool.tile([1, 480], F32, tag="den", name=f"den_h{hi}_{ci}")
                    nc.vector.tensor_copy(den[0:1, 0:qn], o_ps[64:65, 0:qn])
                    dent = epool.tile([96, 5], F32, tag="dent", name=f"dent_h{hi}_{ci}")
                    nr = qn // 96
                    nc.sync.dma_start(dent[0:96, 0:nr], den[0:1, 0:qn])
                    rect = epool.tile([96, 5], F32, tag="rect", name=f"rect_h{hi}_{ci}")
                    nc.vector.reciprocal(rect[0:96, 0:nr], dent[0:96, 0:nr])
                    recb = epool.tile([96, 5], BF16, tag="recb", name=f"recb_h{hi}_{ci}")
                    nc.vector.tensor_copy(recb[0:96, 0:nr], rect[0:96, 0:nr])
                    recr = epool.tile([1, 480], BF16, tag="recr", name=f"recr_h{hi}_{ci}")
                    nc.sync.dma_start(recr[0:1, 0:qn], recb[0:96, 0:nr])
                    return recr

                def epilogue_b(ci, q0, qn, o_ps, recr, hi=hi, c0=c0, c1=c1):
                    # part B: broadcast 1/den over 64 rows, scale, store
                    rb_ps = ps_s.tile([128, 3, 512], F32, tag="s", name=f"rb_h{hi}_{ci}")
                    nc.tensor.matmul(
                        rb_ps[0:64, 0, 0:qn], ones1[:], recr[0:1, 0:qn],
                        start=True, stop=True,
                    )
                    rb_sb = epool.tile([64, 480], F32, tag="rb_sb", name=f"rbs_h{hi}_{ci}")
                    nc.vector.tensor_copy(rb_sb[:, 0:qn], rb_ps[0:64, 0, 0:qn])
                    ot = epool.tile([64, 480], F32, tag="ot", name=f"ot_h{hi}_{ci}")
                    nc.vector.tensor_mul(
                        ot[:, 0:qn], o_ps[0:64, 0:qn], rb_sb[:, 0:qn]
                    )
                    nc.sync.dma_start(out_t[c0:c1, q0 : q0 + qn], ot[:, 0:qn])

                def flush_pend(last_mm, keep):
                    while len(pend) > keep:
                        (ci, q0, qn, o_ps, g, p2) = pend.pop(0)
                        for j in range(3):
                            kt = 3 * g + j
                            pv = nc.tensor.matmul(
                                o_ps[0:65, 0:qn],
                                vt[:, kt * 65 : (kt + 1) * 65],
                                p2[:, j, 0:qn],
                                start=(kt == 0), stop=(kt == KT - 1),
                            )
                            if last_mm is not None:
                                add_dep_helper(pv.ins, last_mm.ins, sync=False,
                                               reason="pv after score mms")
                        if g == NG - 1:
                            recr = epilogue_a(ci, q0, qn, o_ps)
                            epiB.append((ci, q0, qn, o_ps, recr, gctr[0] + 3))
                    while epiB and epiB[0][5] <= gctr[0]:
                        (ci, q0, qn, o_ps, recr, _) = epiB.pop(0)
                        epilogue_b(ci, q0, qn, o_ps, recr)

                for ci, (q0, qn) in enumerate(QCHUNKS):
                    o_ps = ps_o.tile([65, 512], F32, tag="o", name=f"o_h{hi}_{ci}")
                    for g in range(NG):
                        s_ps = ps_s.tile([128, 3, 512], F32, tag="s",
                                         name=f"s_h{hi}_{ci}_{g}")
                        last_mm = None
                        for j in range(3):
                            kt = 3 * g + j
                            last_mm = nc.tensor.matmul(
                                s_ps[:, j, 0:qn],
                                lhsT[:, kt * 128 : (kt + 1) * 128],
                                rhs[:, q0 : q0 + qn],
                                start=True, stop=True,
                            )
                        gctr[0] += 1
                        flush_pend(last_mm, PV_LAG - 1)
                        p1 = p1pool.tile([128, 3, 480], BF16, tag="p1")
                        nc.scalar.activation(p1[:, :, 0:qn], s_ps[:, :, 0:qn], Exp)
                        p2 = p2pool.tile([128, 3, 480], BF16, tag="p2")
                        nc.vector.tensor_mul(
                            p2[:, :, 0:qn], p1[:, :, 0:qn], pats[:, :, q0 : q0 + qn]
                        )
                        pend.append((ci, q0, qn, o_ps, g, p2))
                gctr[0] += 3
                flush_pend(None, 0)
                gctr[0] += 10
                flush_pend(None, 0)

    nc.compile()
    return nc


def _get_nc():
    global _NC
    if _NC is None:
        _NC = _build_nc()
    return _NC


def _host_prep(q, k, v, rel_pos_h, rel_pos_w):
    q2 = np.asarray(q, np.float32).reshape(HW, NH * DH)
    k2 = np.asarray(k, np.float32).reshape(HW, NH * DH)
    v2 = np.asarray(v, np.float32).reshape(HW, NH * DH)
    rph = np.asarray(rel_pos_h, np.float32)
    rpw = np.asarray(rel_pos_w, np.float32)

    ar = np.arange(48)
    # reversed rel tables, x8 cancels the 0.125 q scale
    rhv = np.ascontiguousarray((8.0 * rph[::-1]).T).astype(BF)   # (64, 95)
    rwv = np.ascontiguousarray((8.0 * rpw[::-1]).T).astype(BF)
    kk = np.arange(HW)
    eh = np.zeros((64, HW), np.float32)
    eh[:48] = kk[None, :] // 48 == ar[:, None]
    eh = eh.astype(BF)

    onecol = np.ones((HW, 1), np.float32)
    in_maps = []
    for c in range(N_CORES):
        sl = slice(c * 128, (c + 1) * 128)
        qs = (q2[:, sl].T * 0.125).astype(BF)                    # (128, HW)
        qw = np.ascontiguousarray(
            qs.reshape(128, 48, 48).transpose(0, 2, 1)
        ).reshape(128, HW)                                       # w-major cols
        ks = k2[:, sl].T.astype(BF)
        vparts = []
        for hh in range(HPC):
            vh = v2[:, c * 128 + hh * 64 : c * 128 + (hh + 1) * 64]
            va = np.concatenate([vh, onecol], axis=1)            # (HW, 65)
            vparts.append(va.reshape(KT, 128, 65).transpose(1, 0, 2).reshape(128, KT * 65))
        v_til = np.concatenate(vparts, axis=1).astype(BF)        # (128, 2*18*65)
        in_maps.append(
            dict(q_t=qs, qw_t=qw, k_t=ks, v_til=v_til, rhv=rhv, rwv=rwv, eh=eh)
        )
    return in_maps


def _assemble(results):
    cols = [np.asarray(r["out_t"], np.float32).T for r in results]  # (HW, 128) each
    return np.concatenate(cols, axis=1).reshape(1, H, W, NH * DH)


def kernel(q, k, v, rel_pos_h, rel_pos_w):
    nc = _get_nc()
    in_maps = _host_prep(q, k, v, rel_pos_h, rel_pos_w)
    res = run_bass_kernel_spmd(nc, in_maps, core_ids=list(range(N_CORES)))
    return _assemble(res.results)
